# revision 30
# baseline (speedup 1.0000x reference)
"""BatchedLIDIA kNN patch-denoise kernel for 8 Trainium2 NeuronCores.

Reformulation (validated vs reference in numpy, rel err ~2.5e-5):
  - patch distance d = Nq + Nn - 2*XCorr where XCorr is a 5x5 box-sum of
    per-offset shifted image cross-products; N = box-sum of |patch|^2.
  - softmax weight (per-pixel constant exp(Nq) cancels): w' = exp(2*XC - Nn)
  - top-14 selection by thresholding w' at its 14th-largest value per pixel
    (vector.max8 + match_replace + max8).
  - gather+fold collapses to: acc[u,v,c] = sum_o box5(what_o)[u,v] *
    xp[u+oy, v+ox, c]  (no index gather needed).
Sharding: 8 cores = 4 frames x 2 row-halves (64 query rows each + 2-row fold
halo). All spatial row shifts are DMA copies / PE band-matmuls (engines
require partition-aligned operands); the pixel-major shear for top-k uses PE
transposes.
"""
import numpy as np
import ml_dtypes

import concourse.bass as bass
import concourse.mybir as mybir
import concourse.tile as tile
from concourse.bass_utils import run_bass_kernel_spmd

F32 = mybir.dt.float32
BF16 = mybir.dt.bfloat16
AF = mybir.ActivationFunctionType
OP = mybir.AluOpType

PS, K, WS = 5, 14, 29
# Hardware's f32->u8 activation convert rounds to nearest (CoreSim truncates
# instead — sim-only divergence, so simcheck.py reads ~4e-3 while HW ~2e-3).
_ROUND_BIAS = np.float32(0.0)
H = W = 128
C = 3
PH, WH = 2, 14
QN = 68      # query rows incl +-2 fold halo
XR = 100     # slab rows
XC = 160     # slab cols
GN = 96      # EN-map rows needed per core
NCORES = 8

_PROGRAM = None


def ap(t, part, dims, elem_off=0):
    """Build an AP on tile t: part=(p0, np); dims=[(step, count), ...] in elems."""
    fs = 1
    for s in t.shape[1:]:
        fs *= s
    return bass.AP(tensor=t.tensor, offset=part[0] * fs + elem_off,
                   ap=[(fs, part[1])] + list(dims))


def split_multi_waits(nc):
    """This container's walrus accepts one on_wait per instruction; hoist
    extras onto engine NoOps inserted just before (same engine, in order)."""
    n = 0
    for fn in nc.m.functions:
        for bb in fn.blocks:
            new_list = []
            for ins in bb.instructions:
                si = ins.sync_info
                if si is not None and si.on_wait is not None and len(si.on_wait) > 1:
                    waits = list(si.on_wait)
                    for w in waits[:-1]:
                        n += 1
                        new_list.append(mybir.InstNoOp(
                            name=f"I-{nc.next_id()}",
                            engine=ins.engine,
                            sync_info=mybir.SyncInfo(on_wait=[w], on_update=[]),
                        ))
                    si.on_wait = [waits[-1]]
                new_list.append(ins)
            bb.instructions = new_list
    return n


def build_program():
    nc = bass.Bass()
    d_slab = nc.dram_tensor("xslab", [XR, C, 128], BF16, kind="ExternalInput")
    d_affp = nc.dram_tensor("affp", [64, 4], F32, kind="ExternalInput")
    d_invc = nc.dram_tensor("invc", [64, 132], F32, kind="ExternalInput")
    d_mrow = nc.dram_tensor("mrowp", [128, QN], F32, kind="ExternalInput")
    d_b5a = nc.dram_tensor("b5a", [72, QN], BF16, kind="ExternalInput")
    d_b5b = nc.dram_tensor("b5b", [QN, 64], BF16, kind="ExternalInput")
    d_b5n = nc.dram_tensor("b5n", [XR, GN], F32, kind="ExternalInput")
    d_id68 = nc.dram_tensor("id68", [QN, QN], BF16, kind="ExternalInput")
    d_id128 = nc.dram_tensor("id128", [128, 128], BF16, kind="ExternalInput")
    d_acc = nc.dram_tensor("accd", [64, C * 128], mybir.dt.uint8,
                           kind="ExternalOutput")

    with tile.TileContext(nc) as tc:
        with tc.tile_pool(name="main", bufs=1) as mp:
            t_slab_bf = mp.tile([XR, C, XC], BF16)
            nc.sync.dma_start(t_slab_bf[:, :, WH + 2:WH + 2 + 128], d_slab[:])
            # reflect-pad columns on device: col j<16 <- col 32-j ; col 144+k <- 142-k
            nc.scalar.activation(
                ap(t_slab_bf, (0, XR), [(XC, C), (1, 16)], elem_off=0),
                ap(t_slab_bf, (0, XR), [(XC, C), (-1, 16)], elem_off=32), AF.Copy)
            nc.scalar.activation(
                ap(t_slab_bf, (0, XR), [(XC, C), (1, 16)], elem_off=144),
                ap(t_slab_bf, (0, XR), [(XC, C), (-1, 16)], elem_off=142), AF.Copy)
            t_slab = mp.tile([XR, C, XC], F32)
            nc.scalar.activation(t_slab[:], t_slab_bf[:], AF.Copy)
            t_slabq = mp.tile([72, C, XC], F32)
            nc.sync.dma_start(t_slabq[:], t_slab[14:14 + 72, :, :])
            t_mrow = mp.tile([128, QN], F32)
            nc.sync.dma_start(t_mrow[:], d_mrow[:])
            t_b5a = mp.tile([72, QN], BF16)
            nc.sync.dma_start(t_b5a[:], d_b5a[:])
            t_b5b = mp.tile([QN, 64], BF16)
            nc.sync.dma_start(t_b5b[:], d_b5b[:])
            t_b5n = mp.tile([XR, GN], F32)
            nc.sync.dma_start(t_b5n[:], d_b5n[:])
            t_id68 = mp.tile([QN, QN], BF16)
            nc.sync.dma_start(t_id68[:], d_id68[:])
            t_id128 = mp.tile([128, 128], BF16)
            nc.sync.dma_start(t_id128[:], d_id128[:])

            t_wpix = mp.tile([128, QN, WS, WS], BF16)  # pixel-major weights
            t_en = mp.tile([GN, 156], F32)
            t_acc = mp.tile([64, C, 132], F32)
            nc.vector.memset(t_acc[:], 0.0)
            t_wsum = mp.tile([128, QN], F32)
            t_rw = mp.tile([128, QN], F32)
            t_rm = mp.tile([128, QN], BF16)

            # ---- setup: S, N, EN maps ----
            with tc.tile_pool(name="setup", bufs=1) as sp, \
                 tc.tile_pool(name="setup_ps", bufs=1, space="PSUM") as spp:
                tS = sp.tile([XR, XC], F32)
                tmpS = sp.tile([XR, XC], F32)
                nc.vector.tensor_mul(tS[:], t_slab[:, 0, :], t_slab[:, 0, :])
                nc.vector.tensor_mul(tmpS[:], t_slab[:, 1, :], t_slab[:, 1, :])
                nc.vector.tensor_add(tS[:], tS[:], tmpS[:])
                nc.vector.tensor_mul(tmpS[:], t_slab[:, 2, :], t_slab[:, 2, :])
                nc.vector.tensor_add(tS[:], tS[:], tmpS[:])
                tSh = sp.tile([XR, XC], F32)
                nc.vector.tensor_add(tmpS[:, 0:159], tS[:, 0:159], tS[:, 1:160])
                nc.vector.tensor_add(tSh[:, 0:157], tmpS[:, 0:157], tmpS[:, 2:159])
                nc.vector.tensor_add(tSh[:, 0:156], tSh[:, 0:156], tS[:, 4:160])
                pN = spp.tile([GN, 156], F32)
                nc.tensor.matmul(pN[:], t_b5n[:], tSh[:, 0:156], start=True, stop=True)
                nc.scalar.activation(t_en[:], pN[:], AF.Exp, scale=-1.0)

            # ---- phase 1: weights w' per sy, sheared into t_wpix ----
            with tc.tile_pool(name="p1", bufs=2) as p1, \
                 tc.tile_pool(name="p1a", bufs=1) as p1a, \
                 tc.tile_pool(name="p1d", bufs=2, space="PSUM") as p1d, \
                 tc.tile_pool(name="p1s", bufs=1, space="PSUM") as p1s:
                for sy in range(WS):
                    xqs = p1.tile([72, C, XC], F32, tag="xqs")
                    nc.sync.dma_start(xqs[:], t_slab[sy:sy + 72, :, :])
                    enn = p1.tile([QN, 156], F32, tag="enn")
                    nc.sync.dma_start(enn[:], t_en[sy:sy + QN, :])

                    xx = p1a.tile([72, 132, WS], BF16, tag="xx")
                    tmp = p1a.tile([72, 132, WS], BF16, tag="tmp")
                    for ch in range(C):
                        q_ap = ap(t_slabq, (0, 72), [(1, 132), (0, WS)],
                                  elem_off=ch * XC + 14)
                        n_ap = ap(xqs, (0, 72), [(1, 132), (1, WS)],
                                  elem_off=ch * XC)
                        if ch == 0:
                            nc.vector.tensor_tensor(xx[:], q_ap, n_ap, op=OP.mult)
                        else:
                            nc.vector.tensor_tensor(tmp[:], q_ap, n_ap, op=OP.mult)
                            nc.vector.tensor_add(xx[:], xx[:], tmp[:])
                    # horizontal box5 over x: xxh[x] = sum_dx xx[x+dx]
                    xxh = p1a.tile([72, 129, WS], BF16, tag="xxh")
                    nc.vector.tensor_add(tmp[:, 0:131, :], xx[:, 0:131, :], xx[:, 1:132, :])
                    nc.vector.tensor_add(xxh[:, 0:129, :], tmp[:, 0:129, :], tmp[:, 2:131, :])
                    nc.vector.tensor_add(xxh[:, 0:128, :], xxh[:, 0:128, :], xx[:, 4:132, :])
                    # vertical box5 on PE + exp -> w' ; x in quarters of 32
                    wt = p1.tile([QN, 128, WS], BF16, tag="wt")
                    for qx in range(4):
                        pD = p1d.tile([QN, 2, 512], F32, tag="pD")
                        for j in range(2):
                            x0 = qx * 32 + j * 16
                            nc.tensor.matmul(
                                pD[:, j, 0:16 * WS], t_b5a[:],
                                xxh[:, x0:x0 + 16, :], start=True, stop=True)
                        e2d = p1.tile([QN, 32, WS], F32, tag="e2d")
                        nc.scalar.activation(e2d[:], pD[:, :, 0:16 * WS],
                                             AF.Exp, scale=2.0)
                        en_ap = ap(enn, (0, QN), [(1, 32), (1, WS)], elem_off=qx * 32)
                        nc.vector.tensor_tensor(
                            wt[:, qx * 32:(qx + 1) * 32, :], e2d[:], en_ap, op=OP.mult)
                    # shear via PE transposes: [QN,128] plane per sx -> [128,QN]
                    for g0, gn in ((0, 15), (15, 14)):
                        pT = p1s.tile([128, 15, 128], BF16, tag="pT")
                        for i in range(gn):
                            sx = g0 + i
                            in_ap = ap(wt, (0, QN), [(WS, 128)], elem_off=sx)
                            nc.tensor.transpose(pT[:, i, 0:QN], in_ap, t_id68[:])
                        out_ap = ap(t_wpix, (0, 128), [(1, gn), (WS * WS, QN)],
                                    elem_off=sy * WS + g0)
                        in_ap = ap(pT, (0, 128), [(128, gn), (1, QN)])
                        nc.scalar.activation(out_ap, in_ap, AF.Copy)

            # ---- phase 2: top-14 threshold, Wsum, normalize ----
            with tc.tile_pool(name="p2", bufs=2) as p2:
                for q in range(QN):
                    wsl = t_wpix[:, q:q + 1, :, :].rearrange("p a b c -> p (a b c)")
                    m8a = p2.tile([128, 8], BF16, tag="m8a")
                    nc.vector.max(out=m8a[:], in_=wsl)
                    scr = p2.tile([128, WS * WS], BF16, tag="scr")
                    nc.vector.match_replace(out=scr[:], in_to_replace=m8a[:],
                                            in_values=wsl, imm_value=-1.0)
                    m8b = p2.tile([128, 8], BF16, tag="m8b")
                    nc.vector.max(out=m8b[:], in_=scr[:])
                    nc.vector.scalar_tensor_tensor(
                        wsl, wsl, m8b[:, 5:6], wsl,
                        op0=OP.is_ge, op1=OP.mult,
                        accum_out=t_wsum[:, q:q + 1])
                nc.vector.reciprocal(t_rw[:], t_wsum[:])
                nc.vector.tensor_mul(t_rm[:], t_rw[:], t_mrow[:])
                rm_ap = ap(t_rm, (0, 128), [(1, QN), (0, WS), (0, WS)])
                nc.vector.tensor_tensor(t_wpix[:], t_wpix[:], rm_ap, op=OP.mult)

            # ---- phase 3: unshear, box5, accumulate ----
            with tc.tile_pool(name="p3", bufs=2) as p3, \
                 tc.tile_pool(name="p3a", bufs=1) as p3a, \
                 tc.tile_pool(name="p3u", bufs=1, space="PSUM") as p3u, \
                 tc.tile_pool(name="p3g", bufs=1, space="PSUM") as p3g:
                for sy in range(WS):
                    xqs3 = p3.tile([64, C, XC], F32, tag="xqs3")
                    nc.sync.dma_start(xqs3[:], t_slab[sy + 4:sy + QN, :, :])
                    wh = p3a.tile([QN, 136, WS], BF16, tag="wh")
                    nc.vector.memset(wh[:, 0:4, :], 0.0)
                    nc.vector.memset(wh[:, 132:136, :], 0.0)
                    for g0, gn in ((0, 15), (15, 14)):
                        pU = p3u.tile([QN, 15, 128], BF16, tag="pU")
                        for i in range(gn):
                            sx = g0 + i
                            in_ap = ap(t_wpix, (0, 128), [(WS * WS, QN)],
                                       elem_off=sy * WS + sx)
                            nc.tensor.transpose(pU[:, i, :], in_ap, t_id128[:])
                        out_ap = ap(wh, (0, QN), [(1, gn), (WS, 128)],
                                    elem_off=4 * WS + g0)
                        in_ap = ap(pU, (0, QN), [(128, gn), (1, 128)])
                        nc.scalar.activation(out_ap, in_ap, AF.Copy)
                    # horizontal box (with zero pads): gh[vl] = sum_px wh[vl+4-px]
                    t1 = p3a.tile([QN, 136, WS], BF16, tag="t1")
                    gh = p3a.tile([QN, 132, WS], BF16, tag="gh")
                    nc.vector.tensor_add(t1[:, 1:136, :], wh[:, 1:136, :], wh[:, 0:135, :])
                    nc.vector.tensor_add(t1[:, 3:136, :], t1[:, 3:136, :], t1[:, 1:134, :])
                    nc.vector.tensor_add(gh[:, 0:132, :], t1[:, 4:136, :], wh[:, 0:132, :])
                    # vertical box on PE per third (44 vl), evac, mult, reduce, acc
                    for t3 in range(3):
                        v0 = t3 * 44
                        pG = p3g.tile([64, 3, 512], F32, tag="pG")
                        for j, (dv, nv) in enumerate(((0, 16), (16, 16), (32, 12))):
                            nc.tensor.matmul(
                                pG[:, j, 0:nv * WS], t_b5b[:],
                                gh[:, v0 + dv:v0 + dv + nv, :], start=True, stop=True)
                        gs = p3a.tile([64, 44, WS], F32, tag="gs")
                        for j, (dv, nv) in enumerate(((0, 16), (16, 16), (32, 12))):
                            nc.scalar.activation(gs[:, dv:dv + nv, :],
                                                 pG[:, j, 0:nv * WS], AF.Copy)
                        m3 = p3a.tile([64, C, 44, WS], F32, tag="m3")
                        g_ap = ap(gs, (0, 64), [(0, C), (WS, 44), (1, WS)])
                        x_ap = ap(xqs3, (0, 64), [(XC, C), (1, 44), (1, WS)],
                                  elem_off=v0)
                        nc.gpsimd.tensor_tensor(m3[:], g_ap, x_ap, op=OP.mult)
                        red = p3a.tile([64, C, 44], F32, tag="red")
                        nc.vector.tensor_reduce(red[:], m3[:], axis=mybir.AxisListType.X,
                                                op=OP.add)
                        nc.vector.tensor_add(t_acc[:, :, v0:v0 + 44],
                                             t_acc[:, :, v0:v0 + 44], red[:])
            # finalize on device: pixels = clamp(acc*A*invcnt + B, 0, 255) -> u8
            # A = 127.5/sqb and B = 127.5*(mean_x+1) arrive via affp per call
            t_aff = mp.tile([64, 4], F32)
            nc.sync.dma_start(t_aff[:], d_affp[:])
            t_invc = mp.tile([64, 132], F32)
            nc.sync.dma_start(t_invc[:], d_invc[:])
            t_fin = mp.tile([64, C, 132], F32)
            nc.vector.tensor_tensor(
                t_fin[:], t_acc[:],
                ap(t_aff, (0, 64), [(0, C), (0, 132)], elem_off=0), op=OP.mult)
            nc.vector.tensor_tensor(
                t_fin[:], t_fin[:],
                ap(t_invc, (0, 64), [(0, C), (1, 132)]), op=OP.mult)
            nc.vector.tensor_tensor(
                t_fin[:], t_fin[:],
                ap(t_aff, (0, 64), [(1, C), (0, 132)], elem_off=1), op=OP.add)
            nc.vector.tensor_scalar(t_fin[:], t_fin[:], 0.0, 255.0,
                                    op0=OP.max, op1=OP.min)
            t_u8 = mp.tile([64, C, 128], mybir.dt.uint8)
            nc.scalar.activation(t_u8[:], t_fin[:, :, 2:130], AF.Copy)
            nc.sync.dma_start(d_acc[:], t_u8[:].rearrange("p a b -> p (a b)"))
    nsp = split_multi_waits(nc)
    print(f"split_multi_waits: {nsp} nops inserted")
    return nc


_EXEC = None


def _get_exec(nc):
    """Memoized jax.jit(shard_map) executor for the prebuilt module.

    No donation: the bass program fully overwrites its output, so the
    zero-init output operands can live on device permanently and be
    reused every call (saves one h2d per call over the axon tunnel)."""
    global _EXEC
    if _EXEC is not None:
        return _EXEC
    import jax
    from jax.sharding import Mesh, PartitionSpec
    from jax.experimental.shard_map import shard_map
    from concourse import bass2jax
    bass2jax.install_neuronx_cc_hook()
    pname = nc.partition_id_tensor.name if nc.partition_id_tensor else None
    in_names, out_names, out_avals, zero_shapes = [], [], [], []
    for alloc in nc.m.functions[0].allocations:
        if not isinstance(alloc, mybir.MemoryLocationSet):
            continue
        name = alloc.memorylocations[0].name
        if alloc.kind == "ExternalInput":
            if name != pname:
                in_names.append(name)
        elif alloc.kind == "ExternalOutput":
            out_names.append(name)
            shape = tuple(alloc.tensor_shape)
            dtype = mybir.dt.np(alloc.dtype)
            out_avals.append(jax.core.ShapedArray(shape, dtype))
            zero_shapes.append((shape, dtype))
    n_params = len(in_names)
    all_names = in_names + out_names + ([pname] if pname else [])

    def _body(*args):
        operands = list(args)
        if pname:
            operands.append(bass2jax.partition_id_tensor())
        outs = bass2jax._bass_exec_p.bind(
            *operands, out_avals=tuple(out_avals), in_names=tuple(all_names),
            out_names=tuple(out_names), lowering_input_output_aliases=(),
            sim_require_finite=True, sim_require_nnan=True, nc=nc)
        return tuple(outs)

    devices = jax.devices()[:NCORES]
    mesh = Mesh(np.asarray(devices), ("core",))
    specs = (PartitionSpec("core"),) * (n_params + len(out_names))
    fn = shard_map(_body, mesh=mesh, in_specs=specs,
                   out_specs=(PartitionSpec("core"),) * len(out_names),
                   check_rep=False)
    sh = jax.sharding.NamedSharding(mesh, PartitionSpec("core"))
    shapes_by_name = {}
    for alloc in nc.m.functions[0].allocations:
        if not isinstance(alloc, mybir.MemoryLocationSet):
            continue
        if alloc.kind in ("ExternalInput", "ExternalOutput"):
            shapes_by_name[alloc.memorylocations[0].name] = (
                tuple(alloc.tensor_shape), mybir.dt.np(alloc.dtype))
    arg_avals = [
        jax.ShapeDtypeStruct((NCORES * s[0], *s[1:]), d, sharding=sh)
        for s, d in (shapes_by_name[n] for n in in_names + out_names)]

    def _compile():
        return jax.jit(fn, keep_unused=True).lower(*arg_avals).compile()
    try:
        sharded = bass2jax.fast_dispatch_compile(_compile)
    except Exception:
        sharded = jax.jit(fn, keep_unused=True)
    _EXEC = (sharded, in_names, out_names, out_avals, zero_shapes)
    return _EXEC


def _host_prep(noisy, sigma):
    """Fused channel-first prep: slab values = (noisy - mean)*(sqb/127.5),
    reflect-padded to [t, C, 160, 160] bf16. Returns (xq, means_x, sqb)."""
    D = PS * PS * C
    sig = (np.float32(sigma) / 255.0) / np.float32(0.5)
    beta = np.float32(1.0 / (2.0 * sig * sig * D))
    sqb = np.float32(np.sqrt(beta))
    noisy = np.ascontiguousarray(noisy, np.float32)
    m = noisy.mean((-2, -1), keepdims=True)          # [t,C,1,1] raw mean
    means_x = m * np.float32(1.0 / 127.5) - 1.0      # per-frame channel mean of x
    scaled = ((noisy - m) * np.float32(sqb / 127.5)).astype(ml_dtypes.bfloat16)
    pad = PH + WH
    # rows padded on host; column reflection happens on device
    xq = np.pad(scaled, ((0, 0), (0, 0), (pad, pad), (0, 0)), mode="reflect")
    return xq, means_x, sqb


def _const_inputs():
    b5a = np.zeros((72, QN), ml_dtypes.bfloat16)
    for q in range(QN):
        b5a[q:q + 5, q] = 1.0
    b5b = np.zeros((QN, 64), ml_dtypes.bfloat16)
    for u in range(64):
        b5b[u:u + 5, u] = 1.0
    b5n = np.zeros((XR, GN), np.float32)
    for u in range(GN):
        b5n[u:u + 5, u] = 1.0
    id68 = np.eye(QN, dtype=ml_dtypes.bfloat16)
    id128 = np.eye(128, dtype=ml_dtypes.bfloat16)
    return dict(b5a=b5a, b5b=b5b, b5n=b5n, id68=id68, id128=id128)


_STATE = None


def _ensure_state():
    """One-time: build program + executor, park all static operands on
    device (consts, row masks, zero-init output buffers), warm up once.
    Steady-state calls then pay a single axon round trip: async h2d of
    the image slab -> async execute -> one blocking output fetch."""
    global _PROGRAM, _STATE
    if _STATE is not None:
        return _STATE
    import jax
    from jax.sharding import Mesh, PartitionSpec, NamedSharding
    if _PROGRAM is None:
        _PROGRAM = build_program()
    sharded, in_names, out_names, out_avals, zero_shapes = _get_exec(_PROGRAM)
    cnt = np.minimum(np.minimum(np.arange(132) + 1, 132 - np.arange(132)), PS
                     ).astype(np.float32)
    cnt2 = cnt[:, None] * cnt[None, :]
    mrows, invcs = [], []
    for cid in range(NCORES):
        half = cid % 2
        q0 = half * 64 - 2
        mrow = np.zeros((128, QN), np.float32)
        v0, v1 = max(0, -q0), min(QN, H - q0)
        mrow[:, v0:v1] = 1.0
        mrows.append(mrow)
        invc = np.zeros((64, 132), np.float32)
        invc[:, 2:130] = 1.0 / cnt2[half * 64 + 2:half * 64 + 66, 2:130]
        invcs.append(invc)
    static_np = {"mrowp": np.concatenate(mrows, axis=0),
                 "invc": np.concatenate(invcs, axis=0)}
    for k, v in _const_inputs().items():
        static_np[k] = np.concatenate([v] * NCORES, axis=0)
    mesh = Mesh(np.asarray(jax.devices()[:NCORES]), ("core",))
    sh = NamedSharding(mesh, PartitionSpec("core"))
    dev_static = {k: jax.device_put(v, sh) for k, v in static_np.items()}
    dev_zeros = [jax.device_put(np.zeros((NCORES * s[0], *s[1:]), d), sh)
                 for s, d in zero_shapes]
    jax.block_until_ready(list(dev_static.values()) + dev_zeros)
    arg_template = [None if n in ("xslab", "affp") else dev_static[n]
                    for n in in_names] + dev_zeros
    state = dict(sharded=sharded, in_names=in_names, out_names=out_names,
                 out_avals=out_avals, dev_static=dev_static,
                 dev_zeros=dev_zeros, sh=sh,
                 arg_template=arg_template,
                 slab_idx=in_names.index("xslab"),
                 affp_idx=in_names.index("affp"))
    # warm up (traces jit, caches executable, touches NEFF load path)
    dummy = np.zeros((NCORES * XR, C, 128), ml_dtypes.bfloat16)
    dummy_aff = np.zeros((NCORES * 64, 4), np.float32)
    _dispatch(state, dummy, dummy_aff)
    _STATE = state
    return state


def _dispatch(state, slab_concat, affp_concat):
    import jax
    dev_slab = jax.device_put(slab_concat, state["sh"])
    dev_affp = jax.device_put(affp_concat, state["sh"])
    args = state["arg_template"]
    args[state["slab_idx"]] = dev_slab
    args[state["affp_idx"]] = dev_affp
    out_arrs = state["sharded"](*args)
    return np.asarray(out_arrs[0])


def run(noisy, sigma, trace=False):
    import time
    noisy = np.asarray(noisy)
    sigma = int(np.asarray(sigma))
    t = noisy.shape[0]
    state = _ensure_state()
    xq, means_x, sqb = _host_prep(noisy, sigma)
    slab_concat = np.zeros((NCORES * XR, C, 128), ml_dtypes.bfloat16)
    for cid in range(NCORES):
        f, half = cid // 2, cid % 2
        q0 = half * 64 - 2
        lo, hi = max(0, q0), min(160, q0 + XR)
        slab_concat[cid * XR + lo - q0:cid * XR + hi - q0] = \
            xq[f][:, lo:hi].transpose(1, 0, 2)
    # affine params: out = clamp(acc*(127.5/sqb)*invcnt + 127.5*(mean_x+1))
    A = np.float32(127.5 / sqb)
    affp = np.empty((NCORES * 64, 4), np.float32)
    for cid in range(NCORES):
        f = cid // 2
        affp[cid * 64:(cid + 1) * 64, 0] = A
        affp[cid * 64:(cid + 1) * 64, 1:4] = (
            np.float32(127.5) * (means_x[f, :, 0, 0] + 1.0) + _ROUND_BIAS)
    t0 = time.perf_counter()
    u8_all = _dispatch(state, slab_concat, affp)
    exec_s = time.perf_counter() - t0

    class _Res:
        pass
    res = _Res()
    res.exec_time_ns = int(exec_s * 1e9)
    u8_all = u8_all.reshape(NCORES, 64, C, 128)
    out = np.empty((t, C, H, W), np.float32)
    for cid in range(NCORES):
        f, half = cid // 2, cid % 2
        r0 = half * 64
        out[f, :, r0:r0 + 64, :] = \
            u8_all[cid].astype(np.float32).transpose(1, 0, 2)
    return res, out


def kernel(noisy, sigma):
    _, out = run(noisy, sigma, trace=False)
    return out

